# revision 1
# baseline (speedup 1.0000x reference)
"""Trainium2 Bass kernel for nn_ConvSkip (GNN message passing layer).

Computes, for the full graph:
    h    = data @ W_lin                 (b_lin cancels in the laplacian)
    lap  = h - (1/deg) * sum_{j in N(i)} h_j
    out  = relu(lap + merge @ W_tr + b_tr)

Sharding: nodes are sharded round-robin-contiguously across 8 cores.
Each core receives the full `data` tensor ROTATED so its own shard is at
rows [0, SHARD). Every core redundantly computes the full transformed
feature table h (bf16) into its own DRAM, then gathers its own nodes'
neighbor rows with indirect DMA and finishes the layer for its shard.
Neighbor indices are rotated on the host to match the rotated table.
"""

import numpy as np

P = 128
N_NODES = 50000
DEG = 16
D_IN = 128
D_OUT = 64
N_CORES = 8
SHARD = N_NODES // N_CORES  # 6250

# phase-1 supertile: T1 node-tiles of 128 rows per step
T1 = 8
# phase-2 supertile: T2 node-tiles of 128 rows per step
T2 = 7


def _geometry(n_nodes, shard):
    import math

    n_tiles1 = math.ceil(n_nodes / P)
    n_st1 = math.ceil(n_tiles1 / T1)
    pad_n = n_st1 * T1 * P
    n_tiles2 = math.ceil(shard / P)
    n_st2 = math.ceil(n_tiles2 / T2)
    pad_s = n_st2 * T2 * P
    return pad_n, n_st1, pad_s, n_st2


def build_nc(n_nodes=N_NODES, shard=SHARD, deg=DEG, repeat=1):
    import concourse.bass as bass
    import concourse.tile as tile
    from concourse import bacc, mybir
    from concourse.masks import make_identity

    pad_n, n_st1, pad_s, n_st2 = _geometry(n_nodes, shard)

    f32 = mybir.dt.float32
    bf16 = mybir.dt.bfloat16
    i32 = mybir.dt.int32

    nc = bacc.Bacc("TRN2", target_bir_lowering=False)

    data_r = nc.declare_dram_parameter("data_r", [pad_n, D_IN], f32, isOutput=False)
    merge_r = nc.declare_dram_parameter("merge_r", [pad_s, D_IN], f32, isOutput=False)
    idx_r = nc.declare_dram_parameter("idx_r", [pad_s, deg], i32, isOutput=False)
    w_lin = nc.declare_dram_parameter("w_lin", [D_IN, D_OUT], f32, isOutput=False)
    w_tr = nc.declare_dram_parameter("w_tr", [D_IN, D_OUT], f32, isOutput=False)
    b_tr = nc.declare_dram_parameter("b_tr", [D_OUT], f32, isOutput=False)
    out_r = nc.declare_dram_parameter("out_r", [pad_s, D_OUT], f32, isOutput=True)

    with tile.TileContext(nc) as tc:
        with (
            tc.tile_pool(name="const", bufs=1) as cpool,
            tc.tile_pool(name="sbuf", bufs=2) as pool,
            tc.tile_pool(name="ld", bufs=3) as ldpool,
            tc.tile_pool(name="psum", bufs=2, space="PSUM") as psum,
            tc.tile_pool(name="dram", bufs=1, space="DRAM") as dpool,
        ):
            # ---- constants ----
            identity = cpool.tile([P, P], f32)
            make_identity(nc, identity[:])
            w_lin_sb = cpool.tile([P, D_OUT], f32)
            nc.sync.dma_start(out=w_lin_sb[:], in_=w_lin[:, :])
            w_tr_sb = cpool.tile([P, D_OUT], f32)
            nc.sync.dma_start(out=w_tr_sb[:], in_=w_tr[:, :])
            ones1 = cpool.tile([1, P], f32)
            nc.vector.memset(ones1[:], 1.0)
            btr_sb = cpool.tile([1, D_OUT], f32)
            nc.sync.dma_start(out=btr_sb[:], in_=b_tr[None, :])
            btr_t2 = cpool.tile([1, T2, D_OUT], f32)
            nc.vector.tensor_copy(
                out=btr_t2[:], in_=btr_sb[:, None, :].to_broadcast([1, T2, D_OUT])
            )

            # full transformed-feature table in this core's DRAM (bf16)
            h_table = dpool.tile([pad_n, D_OUT], bf16)

            # own-shard h kept in fp32 in SBUF for the skip/laplacian center term
            own_st1 = (pad_s + T1 * P - 1) // (T1 * P)  # how many phase-1 supertiles cover the shard
            h_own = cpool.tile([P, own_st1 * T1, D_OUT], f32)

            def body():
                # ---- phase 1: h table over all nodes ----
                for st in range(n_st1):
                    r0 = st * T1 * P
                    x_sb = ldpool.tile([P, T1, D_IN], f32, tag="ld1")
                    nc.sync.dma_start(
                        out=x_sb[:],
                        in_=data_r[r0 : r0 + T1 * P, :].rearrange("(t p) f -> p t f", p=P),
                    )
                    xT_ps = psum.tile([P, T1, D_IN], f32, tag="tp")
                    for t in range(T1):
                        nc.tensor.transpose(xT_ps[:, t, :], x_sb[:, t, :], identity[:])
                    xT_sb = pool.tile([P, T1, D_IN], f32, tag="xT")
                    if st % 2 == 0:
                        nc.vector.tensor_copy(out=xT_sb[:], in_=xT_ps[:])
                    else:
                        nc.scalar.copy(out=xT_sb[:], in_=xT_ps[:])
                    h_ps = psum.tile([P, T1 * D_OUT], f32, tag="mm")
                    for t in range(T1):
                        nc.tensor.matmul(
                            out=h_ps[:, t * D_OUT : (t + 1) * D_OUT],
                            lhsT=xT_sb[:, t, :],
                            rhs=w_lin_sb[:],
                            start=True,
                            stop=True,
                        )
                    # persist own-shard h in fp32
                    if st < own_st1:
                        if st % 2 == 0:
                            nc.scalar.copy(
                                out=h_own[:, st * T1 : (st + 1) * T1, :],
                                in_=h_ps[:].rearrange("p (t f) -> p t f", t=T1),
                            )
                        else:
                            nc.vector.tensor_copy(
                                out=h_own[:, st * T1 : (st + 1) * T1, :],
                                in_=h_ps[:].rearrange("p (t f) -> p t f", t=T1),
                            )
                    h_sb = pool.tile([P, T1, D_OUT], bf16, tag="hcast")
                    if st % 2 == 0:
                        nc.vector.tensor_copy(
                            out=h_sb[:], in_=h_ps[:].rearrange("p (t f) -> p t f", t=T1)
                        )
                    else:
                        nc.scalar.copy(
                            out=h_sb[:], in_=h_ps[:].rearrange("p (t f) -> p t f", t=T1)
                        )
                    nc.sync.dma_start(
                        out=h_table[r0 : r0 + T1 * P, :].rearrange("(t p) f -> p t f", p=P),
                        in_=h_sb[:],
                    )

                # ---- phase 2: gather + laplacian + skip for own shard ----
                for su in range(n_st2):
                    r0 = su * T2 * P
                    idx_sb = ldpool.tile([P, T2, deg], i32, tag="idx")
                    nc.sync.dma_start(
                        out=idx_sb[:],
                        in_=idx_r[r0 : r0 + T2 * P, :].rearrange("(t p) d -> p t d", p=P),
                    )
                    m_sb = ldpool.tile([P, T2, D_IN], f32, tag="ld1")
                    nc.sync.dma_start(
                        out=m_sb[:],
                        in_=merge_r[r0 : r0 + T2 * P, :].rearrange("(t p) f -> p t f", p=P),
                    )
                    gath = pool.tile([P, T2 * deg * D_OUT], bf16, tag="gath")
                    # HW indirect DMA consumes ONE offset per partition (streams
                    # contiguously per partition), so issue one gather per
                    # (node-tile, neighbor-slot): [128,1] indices -> [128,64] rows.
                    for t in range(T2):
                        for dd in range(deg):
                            k = t * deg + dd
                            nc.gpsimd.indirect_dma_start(
                                out=gath[:, k * D_OUT : (k + 1) * D_OUT],
                                out_offset=None,
                                in_=h_table[:, :],
                                in_offset=bass.IndirectOffsetOnAxis(
                                    ap=idx_sb[:, t, dd : dd + 1], axis=0
                                ),
                            )
                    mT_ps = psum.tile([P, T2, D_IN], f32, tag="tp")
                    for t in range(T2):
                        nc.tensor.transpose(mT_ps[:, t, :], m_sb[:, t, :], identity[:])
                    mT_sb = pool.tile([P, T2, D_IN], f32, tag="xT")
                    if su % 2 == 0:
                        nc.scalar.copy(out=mT_sb[:], in_=mT_ps[:])
                    else:
                        nc.vector.tensor_copy(out=mT_sb[:], in_=mT_ps[:])
                    sk_ps = psum.tile([P, T2 * D_OUT], f32, tag="mm")
                    # bias first: out[m, (t f)] = b_tr[f] via rank-1 matmul with K=1
                    nc.tensor.matmul(
                        out=sk_ps[:],
                        lhsT=ones1[:],
                        rhs=btr_t2[:].rearrange("o t f -> o (t f)"),
                        start=True,
                        stop=False,
                        skip_group_check=True,
                    )
                    for t in range(T2):
                        nc.tensor.matmul(
                            out=sk_ps[:, t * D_OUT : (t + 1) * D_OUT],
                            lhsT=mT_sb[:, t, :],
                            rhs=w_tr_sb[:],
                            start=False,
                            stop=(t == T2 - 1),
                            skip_group_check=True,
                        )
                    nsum = pool.tile([P, T2, D_OUT], f32, tag="nsum")
                    nc.vector.reduce_sum(
                        out=nsum[:],
                        in_=gath[:].rearrange("p (t d f) -> p t f d", t=T2, d=deg),
                        axis=mybir.AxisListType.X,
                    )
                    tmp = pool.tile([P, T2, D_OUT], f32, tag="tmp")
                    nc.vector.scalar_tensor_tensor(
                        out=tmp[:],
                        in0=nsum[:],
                        scalar=-1.0 / deg,
                        in1=h_own[:, su * T2 : (su + 1) * T2, :],
                        op0=mybir.AluOpType.mult,
                        op1=mybir.AluOpType.add,
                    )
                    osum = pool.tile([P, T2, D_OUT], f32, tag="osum")
                    nc.vector.tensor_tensor(
                        out=osum[:],
                        in0=tmp[:],
                        in1=sk_ps[:].rearrange("p (t f) -> p t f", t=T2),
                        op=mybir.AluOpType.add,
                    )
                    orelu = pool.tile([P, T2, D_OUT], f32, tag="orelu")
                    nc.scalar.activation(
                        out=orelu[:],
                        in_=osum[:],
                        func=mybir.ActivationFunctionType.Relu,
                    )
                    nc.sync.dma_start(
                        out=out_r[r0 : r0 + T2 * P, :].rearrange("(t p) f -> p t f", p=P),
                        in_=orelu[:],
                    )


            for _rep in range(repeat):
                body()

    nc.finalize()

    return nc


def _make_in_maps(data, merge, structure, W_lin, W_tr, b_tr, n_nodes, shard):
    pad_n, _, pad_s, _ = _geometry(n_nodes, shard)
    n_cores = n_nodes // shard
    data = np.ascontiguousarray(data, dtype=np.float32)
    merge = np.ascontiguousarray(merge, dtype=np.float32)
    W_lin = np.ascontiguousarray(W_lin, dtype=np.float32)
    W_tr = np.ascontiguousarray(W_tr, dtype=np.float32)
    b_tr = np.ascontiguousarray(b_tr, dtype=np.float32)
    in_maps = []
    for k in range(n_cores):
        lo = k * shard
        d = np.zeros((pad_n, D_IN), dtype=np.float32)
        rolled = np.roll(data, -lo, axis=0)
        d[:n_nodes] = rolled
        m = np.zeros((pad_s, D_IN), dtype=np.float32)
        m[:shard] = merge[lo : lo + shard]
        idx = np.zeros((pad_s, DEG), dtype=np.int32)
        idx[:shard] = ((structure[lo : lo + shard].astype(np.int64) - lo) % n_nodes).astype(
            np.int32
        )
        in_maps.append(
            {
                "data_r": d,
                "merge_r": m,
                "idx_r": idx,
                "w_lin": W_lin,
                "w_tr": W_tr,
                "b_tr": b_tr,
            }
        )
    return in_maps


_NC_CACHE = {}


def _get_nc():
    key = (N_NODES, SHARD)
    if key not in _NC_CACHE:
        _NC_CACHE[key] = build_nc()
    return _NC_CACHE[key]


def kernel(data, merge, structure, W_lin, b_lin, W_tr, b_tr):
    from concourse.bass_utils import run_bass_kernel_spmd

    del b_lin  # cancels exactly in the normalized laplacian
    nc = _get_nc()
    in_maps = _make_in_maps(
        np.asarray(data),
        np.asarray(merge),
        np.asarray(structure),
        np.asarray(W_lin),
        np.asarray(W_tr),
        np.asarray(b_tr),
        N_NODES,
        SHARD,
    )
    res = run_bass_kernel_spmd(nc, in_maps, core_ids=list(range(N_CORES)))
    global LAST_RESULTS
    LAST_RESULTS = res
    out = np.concatenate(
        [np.asarray(res.results[k]["out_r"])[:SHARD] for k in range(N_CORES)], axis=0
    )
    return out.astype(np.float32)



# revision 12
# speedup vs baseline: 1.1047x; 1.1047x over previous
"""Trainium2 Bass kernel for nn_ConvSkip (GNN message passing layer).

Computes, for the full graph:
    h    = data @ W_lin                 (b_lin cancels in the laplacian)
    lap  = h - (1/deg) * sum_{j in N(i)} h_j
    out  = relu(lap + merge @ W_tr + b_tr)

Sharding: nodes are sharded contiguously across 8 cores. Each core receives
the full `data` tensor (bf16, rotated so its own shard is at rows [0, SHARD))
and redundantly computes the full transformed feature table h (bf16) into its
own DRAM. Neighbor rows are fetched with per-column indirect DMAs (128
offsets each, the only dynamic-DMA primitive on this runtime) and the
16-way neighbor sum is done as accumulating selection matmuls on the tensor
engine (host pre-arranges edge order so slot->node mapping is a fixed set of
16 static 0/1 matrices).
"""

import numpy as np

P = 128
N_NODES = 50000
DEG = 16
D_IN = 128
D_OUT = 64
N_CORES = 8
SHARD = N_NODES // N_CORES  # 6250

T1 = 8                      # phase-1 supertile: node-tiles per step
N_ST1 = 49                  # supertiles covering all nodes
PAD_N = N_ST1 * T1 * P      # 50176

CH = 7                      # phase-2: node-tiles per chunk
N_CH = 7                    # chunks covering the shard (49 tiles)
PAD_S = N_CH * CH * P       # 6272
CC = CH * 16                # gather columns per chunk (112)
OWN_ST1 = (PAD_S + T1 * P - 1) // (T1 * P)  # 7 phase-1 supertiles cover shard


def build_nc(repeat=1):
    import concourse.bass as bass
    import concourse.tile as tile
    from concourse import bacc, mybir
    from concourse.masks import make_identity

    f32 = mybir.dt.float32
    bf16 = mybir.dt.bfloat16
    i32 = mybir.dt.int32

    nc = bacc.Bacc("TRN2", target_bir_lowering=False)

    data_r = nc.declare_dram_parameter("data_r", [PAD_N, D_IN], bf16, isOutput=False)
    merge_r = nc.declare_dram_parameter("merge_r", [PAD_S, D_IN], bf16, isOutput=False)
    idx_r = nc.declare_dram_parameter("idx_r", [P, N_CH * CC], i32, isOutput=False)
    w_lin = nc.declare_dram_parameter("w_lin", [D_IN, D_OUT], bf16, isOutput=False)
    w_tr = nc.declare_dram_parameter("w_tr", [D_IN, D_OUT], bf16, isOutput=False)
    b_tr = nc.declare_dram_parameter("b_tr", [D_OUT], f32, isOutput=False)
    s_base = nc.declare_dram_parameter("s_base", [P, 16 * P], bf16, isOutput=False)
    out_r = nc.declare_dram_parameter("out_r", [PAD_S, D_OUT], f32, isOutput=True)

    with tile.TileContext(nc) as tc:
        with (
            tc.tile_pool(name="const", bufs=1) as cpool,
            tc.tile_pool(name="own", bufs=2) as opool,
            tc.tile_pool(name="sbuf", bufs=2) as pool,
            tc.tile_pool(name="ld", bufs=3) as ldpool,
            tc.tile_pool(name="psum", bufs=2, space="PSUM") as psum,
            tc.tile_pool(name="dram", bufs=2, space="DRAM") as dpool,
        ):
            # ---- constants ----
            identity = cpool.tile([P, P], bf16)
            make_identity(nc, identity[:])
            w_lin_sb = cpool.tile([P, D_OUT], bf16)
            nc.sync.dma_start(out=w_lin_sb[:], in_=w_lin[:, :])
            w_tr_sb = cpool.tile([P, D_OUT], bf16)
            nc.sync.dma_start(out=w_tr_sb[:], in_=w_tr[:, :])
            s_base_sb = cpool.tile([P, 16, P], bf16)
            nc.sync.dma_start(
                out=s_base_sb[:], in_=s_base[:, :].rearrange("p (g m) -> p g m", g=16)
            )
            ones1 = cpool.tile([1, P], f32)
            nc.vector.memset(ones1[:], 1.0)
            btr_sb = cpool.tile([1, D_OUT], f32)
            nc.sync.dma_start(out=btr_sb[:], in_=b_tr[None, :])
            btr_t2 = cpool.tile([1, CH, D_OUT], f32)
            nc.vector.tensor_copy(
                out=btr_t2[:], in_=btr_sb[:, None, :].to_broadcast([1, CH, D_OUT])
            )
            idx_all = cpool.tile([P, N_CH * CC], i32)
            nc.sync.dma_start(out=idx_all[:], in_=idx_r[:, :])

            def body(rep):
                # full transformed-feature table in this core's DRAM (bf16);
                # double-buffered so successive bodies pipeline
                h_table = dpool.tile([PAD_N, D_OUT], bf16, tag="ht")
                # own-shard h kept in fp32 in SBUF for the laplacian center
                h_own = opool.tile([P, OWN_ST1 * T1, D_OUT], f32, tag="hown")

                # ---- phase 1: h table over all nodes ----
                for st in range(N_ST1):
                    r0 = st * T1 * P
                    x_sb = ldpool.tile([P, T1, D_IN], bf16, tag="ld1")
                    nc.sync.dma_start(
                        out=x_sb[:],
                        in_=data_r[r0 : r0 + T1 * P, :].rearrange(
                            "(t p) f -> p t f", p=P
                        ),
                    )
                    xT_ps = psum.tile([P, T1, D_IN], bf16, tag="tp")
                    for t in range(T1):
                        nc.tensor.transpose(xT_ps[:, t, :], x_sb[:, t, :], identity[:])
                    xT_sb = pool.tile([P, T1, D_IN], bf16, tag="xT")
                    if st % 2 == 0:
                        nc.vector.tensor_copy(out=xT_sb[:], in_=xT_ps[:])
                    else:
                        nc.scalar.copy(out=xT_sb[:], in_=xT_ps[:])
                    h_ps = psum.tile([P, 512], f32, tag="mm")
                    for t in range(T1):
                        nc.tensor.matmul(
                            out=h_ps[:, t * D_OUT : (t + 1) * D_OUT],
                            lhsT=xT_sb[:, t, :],
                            rhs=w_lin_sb[:],
                            start=True,
                            stop=True,
                        )
                    if st < OWN_ST1:
                        if st % 2 == 0:
                            nc.scalar.copy(
                                out=h_own[:, st * T1 : (st + 1) * T1, :],
                                in_=h_ps[:].rearrange("p (t f) -> p t f", t=T1),
                            )
                        else:
                            nc.vector.tensor_copy(
                                out=h_own[:, st * T1 : (st + 1) * T1, :],
                                in_=h_ps[:].rearrange("p (t f) -> p t f", t=T1),
                            )
                    h_sb = pool.tile([P, T1, D_OUT], bf16, tag="hcast")
                    if st % 2 == 0:
                        nc.vector.tensor_copy(
                            out=h_sb[:], in_=h_ps[:].rearrange("p (t f) -> p t f", t=T1)
                        )
                    else:
                        nc.scalar.copy(
                            out=h_sb[:], in_=h_ps[:].rearrange("p (t f) -> p t f", t=T1)
                        )
                    nc.sync.dma_start(
                        out=h_table[r0 : r0 + T1 * P, :].rearrange(
                            "(t p) f -> p t f", p=P
                        ),
                        in_=h_sb[:],
                    )

                # ---- phase 2: gather + laplacian + skip, per chunk ----
                for ch in range(N_CH):
                    gath = pool.tile([P, CC, D_OUT], bf16, tag="gath")
                    for c in range(CC):
                        k = ch * CC + c
                        nc.gpsimd.indirect_dma_start(
                            out=gath[:, c, :],
                            out_offset=None,
                            in_=h_table[:, :],
                            in_offset=bass.IndirectOffsetOnAxis(
                                ap=idx_all[:, k : k + 1], axis=0
                            ),
                        )
                    # neighbor-sum via accumulating selection matmuls
                    nsum_ps = psum.tile([P, CH, D_OUT], f32, tag="ns")
                    for g in range(16):
                        nc.tensor.matmul(
                            out=nsum_ps[:, :, :].rearrange("m c f -> m (c f)"),
                            lhsT=s_base_sb[:, g, :],
                            rhs=gath[:, g * CH : (g + 1) * CH, :].rearrange(
                                "p c f -> p (c f)"
                            ),
                            start=(g == 0),
                            stop=(g == 15),
                        )
                    # skip branch
                    m_sb = ldpool.tile([P, CH, D_IN], bf16, tag="ldm")
                    r0 = ch * CH * P
                    nc.sync.dma_start(
                        out=m_sb[:],
                        in_=merge_r[r0 : r0 + CH * P, :].rearrange(
                            "(t p) f -> p t f", p=P
                        ),
                    )
                    mT_ps = psum.tile([P, T1, D_IN], bf16, tag="tp")
                    for t in range(CH):
                        nc.tensor.transpose(mT_ps[:, t, :], m_sb[:, t, :], identity[:])
                    mT_sb = pool.tile([P, T1, D_IN], bf16, tag="xT")
                    if ch % 2 == 0:
                        nc.scalar.copy(out=mT_sb[:, :CH, :], in_=mT_ps[:, :CH, :])
                    else:
                        nc.vector.tensor_copy(out=mT_sb[:, :CH, :], in_=mT_ps[:, :CH, :])
                    sk_ps = psum.tile([P, 512], f32, tag="mm")
                    nc.tensor.matmul(
                        out=sk_ps[:, : CH * D_OUT],
                        lhsT=ones1[:],
                        rhs=btr_t2[:].rearrange("o t f -> o (t f)"),
                        start=True,
                        stop=False,
                        skip_group_check=True,
                    )
                    for t in range(CH):
                        nc.tensor.matmul(
                            out=sk_ps[:, t * D_OUT : (t + 1) * D_OUT],
                            lhsT=mT_sb[:, t, :],
                            rhs=w_tr_sb[:],
                            start=False,
                            stop=(t == CH - 1),
                            skip_group_check=True,
                        )
                    # combine: relu(h_own - nsum/deg + skip)
                    tmp = pool.tile([P, CH, D_OUT], f32, tag="tmp")
                    nc.vector.scalar_tensor_tensor(
                        out=tmp[:],
                        in0=nsum_ps[:],
                        scalar=-1.0 / DEG,
                        in1=h_own[:, ch * CH : (ch + 1) * CH, :],
                        op0=mybir.AluOpType.mult,
                        op1=mybir.AluOpType.add,
                    )
                    osum = pool.tile([P, CH, D_OUT], f32, tag="osum")
                    nc.vector.tensor_tensor(
                        out=osum[:],
                        in0=tmp[:],
                        in1=sk_ps[:, : CH * D_OUT].rearrange("p (t f) -> p t f", t=CH),
                        op=mybir.AluOpType.add,
                    )
                    orelu = pool.tile([P, CH, D_OUT], f32, tag="orelu")
                    nc.scalar.activation(
                        out=orelu[:],
                        in_=osum[:],
                        func=mybir.ActivationFunctionType.Relu,
                    )
                    nc.sync.dma_start(
                        out=out_r[r0 : r0 + CH * P, :].rearrange(
                            "(t p) f -> p t f", p=P
                        ),
                        in_=orelu[:],
                    )

            for _rep in range(repeat):
                body(_rep)

    nc.finalize()
    return nc


def _make_in_maps(data, merge, structure, W_lin, W_tr, b_tr):
    import ml_dtypes

    bf16 = ml_dtypes.bfloat16

    data = np.ascontiguousarray(data, dtype=np.float32)
    merge = np.ascontiguousarray(merge, dtype=np.float32)
    structure = np.asarray(structure, dtype=np.int64)
    W_lin_b = np.ascontiguousarray(W_lin, dtype=np.float32).astype(bf16)
    W_tr_b = np.ascontiguousarray(W_tr, dtype=np.float32).astype(bf16)
    b_tr = np.ascontiguousarray(b_tr, dtype=np.float32)

    # S_all[p, g, m] = 1 iff m == g*8 + p%8 (selection matrices, one per group)
    s_base = np.zeros((P, 16, P), dtype=bf16)
    for p in range(P):
        for g in range(16):
            s_base[p, g, g * 8 + p % 8] = 1.0
    s_base = s_base.reshape(P, 16 * P)

    in_maps = []
    for k in range(N_CORES):
        lo = k * SHARD
        d = np.zeros((PAD_N, D_IN), dtype=bf16)
        d[:N_NODES] = np.roll(data, -lo, axis=0).astype(bf16)
        m = np.zeros((PAD_S, D_IN), dtype=bf16)
        m[:SHARD] = merge[lo : lo + SHARD].astype(bf16)

        idxr = np.zeros((PAD_S, DEG), dtype=np.int64)
        idxr[:SHARD] = (structure[lo : lo + SHARD] - lo) % N_NODES
        # edge reorder: column c = g*CH + t_rel (per chunk), partition
        # p = s*8 + nlo; node = (ch*CH + t_rel)*128 + g*8 + nlo, slot = s
        E = idxr.reshape(N_CH * CH, 16, 8, DEG)       # [t, g, nlo, s]
        E = E.transpose(0, 1, 3, 2)                   # [t, g, s, nlo]
        E = E.reshape(N_CH, CH, 16, P)                # [ch, t_rel, g, p]
        E = E.transpose(0, 2, 1, 3)                   # [ch, g, t_rel, p]
        # idx32[p, ch*CC + c] with c = g*CH + t_rel
        idx32 = (
            E.reshape(N_CH * CC, P).T.astype(np.int32)
        )
        in_maps.append(
            {
                "data_r": d,
                "merge_r": m,
                "idx_r": np.ascontiguousarray(idx32),
                "w_lin": W_lin_b,
                "w_tr": W_tr_b,
                "b_tr": b_tr,
                "s_base": s_base,
            }
        )
    return in_maps


_NC_CACHE = {}


def _get_nc():
    if "nc" not in _NC_CACHE:
        _NC_CACHE["nc"] = build_nc()
    return _NC_CACHE["nc"]


def kernel(data, merge, structure, W_lin, b_lin, W_tr, b_tr):
    from concourse.bass_utils import run_bass_kernel_spmd

    del b_lin  # cancels exactly in the normalized laplacian
    nc = _get_nc()
    in_maps = _make_in_maps(
        np.asarray(data),
        np.asarray(merge),
        np.asarray(structure),
        np.asarray(W_lin),
        np.asarray(W_tr),
        np.asarray(b_tr),
    )
    res = run_bass_kernel_spmd(nc, in_maps, core_ids=list(range(N_CORES)))
    global LAST_RESULTS
    LAST_RESULTS = res
    out = np.concatenate(
        [np.asarray(res.results[k]["out_r"])[:SHARD] for k in range(N_CORES)], axis=0
    )
    return out.astype(np.float32)
